# revision 109
# baseline (speedup 1.0000x reference)
"""TRN2 Bass kernel for nn_Attention_56392920596865.

Structure exploited (B=4, S=2048, D=1024, H=16, HD=64):
  - The "buggy head shuffle" maps chunk (b, s, h) -> shuffled batch b' = s//512,
    so attention for shuffled batch b' only consumes projected rows from input
    sequence window s in [512b', 512(b'+1)), all input batches. Each core
    (bp = c//2 over shuffled batch, qh = c%2 over query halves) computes its own
    Q/K/V projections locally -> no collectives.
  - The second shuffle gives each core exactly 2 of the 16 mh feature blocks for
    ALL output rows -> each core computes a partial o = mh[:, blk] @ W_o[:, blk]^T
    over all 8192 rows and the host sums the 8 partials.
  - All matmuls run in bf16 (same PE rate as fp32r, half the DMA/SBUF traffic;
    end-to-end rel err ~7e-3 vs the 2e-2 gate).
  - Shuffle layout uses a consistent column permutation col' = (h>>2)*nsig + sigma
    of the shuffled k'/q' index so every psum scatter-evict is contiguous; the
    permutation cancels inside the attention contraction sums.
  - Host pre-lays W as [p][j][t][c] and x as [p][t][c] so every DMA descriptor
    moves >=1KB contiguous runs (no sub-512B penalty, minimal descriptor count).
  - All bulk loads ride one queue (sync/SP) in emission order so the serial
    DMA-engine FIFO sees x1a (t 0-3), wk0, x1b (t 4-7), wk1.. exactly when
    needed: j0's first four t-steps start at the x1a+wk0 sem (~4.35us; every
    DMA sem fires wire-end + 900ns).  Tiny constant loads (ones/ident/woT)
    are dep-anchored behind a phase-2 eviction so the Tile scheduler cannot
    hoist them into the critical early wire slots.  PSUM evictions alternate
    ACT/DVE (GPSIMD cannot read PSUM; DMA cannot touch PSUM).  Output stores
    (r0<3) stage 4 row tiles but ship as TWO 2-tile DMAs, each half as soon
    as its evicts land; late halves ride gpsimd so a lagging store never
    head-of-line blocks the next pair's repT2 transposes on the sync queue.
    The final pair's 16 single-tile stores all ride sync: its 625ns HWDGE
    gen (vs SWDGE's 1038+650) is what keeps the 728ns/tile end-wire cadence
    fed.  V''^T -> V'' and rep -> repT transposes
    use the XBAR DMA-transpose engine (16x128 tiles, bf16) instead of the PE,
    except the final rep pair which stays on the PE to cut tail latency.
    A 25-matmul warmup ramps the PE p-state while the first DMAs land, and
    phase-6 output matmuls interleave into the AV accumulation loops (2/kt,
    starting only in av(even,1): repT2 XBAR transposes need ~3us to land).
    The final pair runs df-major with av(7,1) as four 128-col chains so the
    hp8-15 transpose->out-proj->evict->store tail pipelines into the last AV.

Per-core phases (one Tile program; phases overlap via emission interleaving):
  1/2. K''^T and Q''^T via projection matmuls with shuffle-scatter psum evicts
  3.   S^T = K''^T.T @ Q''^T (scores transposed), ACT exp((1/32) s) -> expS
  4.   V projection -> V''^T scatter -> PE-transpose -> V'' (k'-natural)
  5.   Z = expS-column matmuls; rep = (expS.T @ V'') / Z written (d,parity)-
       interleaved per qs pair; PE-transpose pairs -> repT2 [(dh,delta), h', r0, m]
  6.   (interleaved with 5) o_part row tiles = repT2 K=128 matmuls against
       host-row-interleaved W_o^T slice; host unscrambles the (h', r0, b, hi)
       row permutation: s = hi*64 + r0*16 + h'.
"""
import sys
import numpy as np

try:
    import concourse.bass  # noqa: F401
except ImportError:
    sys.path.insert(0, "/opt/trn_rl_repo")

B, S, D, H, HD = 4, 2048, 1024, 16, 64

_CACHE = {}


def _build_program():
    from contextlib import ExitStack

    import concourse.mybir as mybir
    import concourse.tile as tile
    from concourse import bacc

    F32 = mybir.dt.float32
    BF16 = mybir.dt.bfloat16
    AFT = mybir.ActivationFunctionType

    nc = bacc.Bacc(None, target_bir_lowering=False, debug=False)

    with tile.TileContext(nc) as tc:
        with tc.tile_pool(name="dram", bufs=1, space="DRAM") as dram:
            # x tensors: [p][t][c] with original row index = t*128+p (transposed
            # window); W tensors: [p][j][t][c] (j = output 128-block).
            kx = dram.tile([128, 8, 2048], BF16, kind="ExternalInput", name="kx", uniquify=False)
            qx = dram.tile([128, 8, 1024], BF16, kind="ExternalInput", name="qx", uniquify=False)
            vx = dram.tile([128, 8, 2048], BF16, kind="ExternalInput", name="vx", uniquify=False)
            wk = dram.tile([128, 8, 8, 128], BF16, kind="ExternalInput", name="wk", uniquify=False)
            wq = dram.tile([128, 8, 8, 128], BF16, kind="ExternalInput", name="wq", uniquify=False)
            wv = dram.tile([128, 8, 8, 128], BF16, kind="ExternalInput", name="wv", uniquify=False)
            woTa = dram.tile([128, 1024], BF16, kind="ExternalInput", name="woTa", uniquify=False)
            ones1 = dram.tile([128, 4], BF16, kind="ExternalInput", name="ones1", uniquify=False)
            ident = dram.tile([128, 128], BF16, kind="ExternalInput", name="ident", uniquify=False)
            o_part = dram.tile([8192, 1024], BF16, kind="ExternalOutput", name="o_part", uniquify=False)

            def load_w_full(pool, w_dram, nm, split=False, skip_first=False):
                w_sb = pool.tile([128, 8, 8, 128], BF16, name=nm, tag="wfull")
                if split:
                    # same queue as the x loads: the sync queue serializes
                    # HWDGE gens, giving the FIFO order wk0, x1, wk1, wk2, ...
                    for lo, hi in ((0, 1), (1, 2), (2, 3), (3, 4), (4, 5), (5, 6),
                                   (6, 7), (7, 8)):
                        if skip_first and lo == 0:
                            continue
                        nc.sync.dma_start(w_sb[:, lo:hi], w_dram[:, lo:hi])
                else:
                    nc.sync.dma_start(w_sb[:], w_dram[:])
                return w_sb

            # Round-robin eviction engines.  Phases 1-3 use ACT/DVE only (the
            # Pool queue is busy streaming weights then); later phases add
            # Pool.
            ev_state = {"i": 0}

            def evict(dst, src, engines):
                e = engines[ev_state["i"] % len(engines)]
                ev_state["i"] += 1
                if e == "v":
                    ev_state["last"] = nc.vector.tensor_copy(dst, src)
                elif e == "s":
                    ev_state["last"] = nc.scalar.copy(dst, src)
                else:
                    ev_state["last"] = nc.gpsimd.tensor_copy(dst, src)

            def scatter_evict(dst_fn, ps, j, gcol0, width, nsig, engines):
                seg = min(nsig, width)
                for hh in (0, 1):
                    h = 2 * j + hh
                    for s_off in range(0, width, seg):
                        gcol = gcol0 + s_off
                        b = gcol // nsig
                        hp = 4 * (h & 3) + b
                        c0 = (h >> 2) * nsig + (gcol % nsig)
                        dst = dst_fn(hp)[64 * (hp & 1):64 * (hp & 1) + 64, c0:c0 + seg]
                        srcp = ps[64 * hh:64 * hh + 64, s_off:s_off + seg]
                        evict(dst, srcp, engines)

            def proj_scatter(dst_fn, x_dram, nsig, blocks, w_sb, stg, psp,
                             preloaded=None):
                """Project x window by W^T; scatter-evict into shuffled-
                transposed dst. blocks = list of (col0, width)."""
                for bl, (c0b, wb) in enumerate(blocks):
                    if bl == 0 and preloaded is not None:
                        x_sb = preloaded
                    else:
                        x_sb = stg.tile([128, 8, 512], BF16, name="x_sb", tag="x_sb",
                                        padded_shape=[128, 8, 512])
                        nc.sync.dma_start(x_sb[:, :, 0:wb], x_dram[:, :, c0b:c0b + wb])
                    engines = ("v", "s")
                    for j in range(8):
                        ps = psp.tile([128, 512], F32, name="ps", tag="ps")
                        for t in range(8):
                            nc.tensor.matmul(ps[:, 0:wb], w_sb[:, j, t],
                                             x_sb[:, t, 0:wb], start=(t == 0), stop=(t == 7))
                        scatter_evict(dst_fn, ps[:, 0:wb], j, c0b, wb, nsig, engines)

            # Warm the PE p-state ramp with throwaway matmuls while the
            # first weight/x DMAs are still in flight (cost model: full speed
            # only after ~3us of continuous PE busy).  Sized so the last
            # warmup matmul ends just AFTER the wk0+x1 DMAs land (~4.15us):
            # any PE idle seam would reset pe_busy_start and hold the first
            # ~3us of projection matmuls at the 1.2GHz mid p-state.
            NWARM = 25
            with tc.tile_pool(name="wrm", bufs=1) as wrm, \
                 tc.tile_pool(name="wrmp", bufs=1, space="PSUM") as wrmp:
                wt = wrm.tile([128, 128], BF16, name="wt")
                nc.vector.memset(wt[:], 0)
                wps = wrmp.tile([128, 128], F32, name="wps", tag="wps")
                for i in range(NWARM):
                    nc.tensor.matmul(wps[:], wt[:], wt[:],
                                     start=(i == 0), stop=(i == NWARM - 1))

            stkKQ = ExitStack()
            pK = stkKQ.enter_context(tc.tile_pool(name="pK", bufs=1))
            K2T = pK.tile([128, 8, 2048], BF16, name="K2T")
            pQ = stkKQ.enter_context(tc.tile_pool(name="pQ", bufs=1))
            Q2T = pQ.tile([128, 8, 1024], BF16, name="Q2T")

            # Right-stack pools that must exist before scores: expS, the
            # transpose identity, V-phase x staging and V weights (prefetched
            # while scores run).
            stkE = ExitStack()
            pE = stkE.enter_context(tc.tile_pool(name="pE", bufs=1, side="right"))
            expS = pE.tile([128, 16, 1024], BF16, name="expS")
            stkI = ExitStack()
            cpool = stkI.enter_context(tc.tile_pool(name="cpool", bufs=1, side="right"))
            id_sb = cpool.tile([128, 128], BF16, name="id_sb")
            wop = stkI.enter_context(tc.tile_pool(name="wop", bufs=1, side="right"))
            wo_a = wop.tile([128, 1024], BF16, name="wo_a")
            cp2 = stkI.enter_context(tc.tile_pool(name="cp2", bufs=1, side="right"))
            ones_sb = cp2.tile([128, 4], BF16, name="ones_sb")
            stkW = ExitStack()
            vstg = stkW.enter_context(tc.tile_pool(name="vstg", bufs=4, side="right"))
            pVw = stkW.enter_context(tc.tile_pool(name="pVw", bufs=1, side="right"))
            stkVT = ExitStack()
            v2t_pool = stkVT.enter_context(
                tc.tile_pool(name="v2t", bufs=4, side="right"))

            # One psum bank in a pool that OUTLIVES the psA->psB swap: the
            # first V-projection group runs from it, so the PE rolls straight
            # from the last scores matmul into V work while the pool boundary
            # drains behind it.  The bank is repaid by hosting the tiny
            # z/transpose staging tiles in phases 5-6 (psB = vps 5 + pa 2).
            stkV0 = ExitStack()
            psV0 = stkV0.enter_context(tc.tile_pool(name="psV0", bufs=1, space="PSUM"))

            # phases 1-3 share one PSUM pool (same tag) so there is no
            # drain/reopen gap between the projections and the scores.
            stkPS = ExitStack()
            psA = stkPS.enter_context(tc.tile_pool(name="psA", bufs=7, space="PSUM"))
            with tc.tile_pool(name="pW", bufs=2) as pW, \
                 tc.tile_pool(name="stp", bufs=3) as stp:
                # FIFO order x1a (t 0-3), wk0, x1b (t 4-7), wk1, ...: j0's
                # t=0..3 matmuls need only x1a+wk0 (wire-end + 900ns sem-prop
                # ~= 4.3us), and x1b lands before the t=4 step comes up.  The
                # warmup is sized to end just after the wk0 sem: any PE idle
                # seam resets pe_busy_start to the 1.2GHz mid p-state.
                x1 = stp.tile([128, 8, 512], BF16, name="x_sb", tag="x_sb",
                              padded_shape=[128, 8, 512])
                w_k = pW.tile([128, 8, 8, 128], BF16, name="w_k", tag="wfull")
                nc.sync.dma_start(x1[:, 0:4, 0:256], kx[:, 0:4, 0:256])
                nc.sync.dma_start(w_k[:, 0:1], wk[:, 0:1])
                nc.sync.dma_start(x1[:, 4:8, 0:256], kx[:, 4:8, 0:256])
                for lo in range(1, 8):
                    nc.sync.dma_start(w_k[:, lo:lo + 1], wk[:, lo:lo + 1])
                proj_scatter(lambda hp: K2T[:, hp >> 1, :], kx, 512,
                             [(0, 256), (256, 256), (512, 512), (1024, 512), (1536, 512)],
                             w_sb=w_k, stg=stp, psp=psA, preloaded=x1)
                w_q = load_w_full(pW, wq, "w_q")
                w_v = load_w_full(pVw, wv, "w_v")
                proj_scatter(lambda hp: Q2T[:, hp >> 1, :], qx, 256,
                             [(0, 512), (512, 512)], w_sb=w_q, stg=stp, psp=psA)

            # phase 3: scores^T + exp.  V x blocks prefetch during scores.
            vx_tiles = []

            def load_vx(bb):
                x_sb = vstg.tile([128, 8, 512], BF16, name="x_sb", tag="vx_sb")
                nc.sync.dma_start(x_sb[:], vx[:, :, bb * 512:(bb + 1) * 512])
                vx_tiles.append(x_sb)

            for bb in range(4):
                load_vx(bb)
            for qb in range(2):
                for kt in range(16):
                    if qb == 1 and kt == 15:
                        # narrow sub-groups: the final exp (gating phase 4 via
                        # PSUM reuse) drains much sooner at free=128
                        for k4 in range(4):
                            ps = psA.tile([128, 512], F32, name="ps_sc", tag="ps")
                            c0 = 512 + k4 * 128
                            for t in range(8):
                                nc.tensor.matmul(
                                    ps[:, 0:128], K2T[:, t, kt * 128:(kt + 1) * 128],
                                    Q2T[:, t, c0:c0 + 128],
                                    start=(t == 0), stop=(t == 7))
                            nc.scalar.activation(expS[:, kt, c0:c0 + 128],
                                                 ps[:, 0:128],
                                                 AFT.Exp, scale=1.0 / 32.0)
                        continue
                    ps = psA.tile([128, 512], F32, name="ps_sc", tag="ps")
                    for t in range(8):
                        nc.tensor.matmul(ps[:], K2T[:, t, kt * 128:(kt + 1) * 128],
                                         Q2T[:, t, qb * 512:(qb + 1) * 512],
                                         start=(t == 0), stop=(t == 7))
                    nc.scalar.activation(expS[:, kt, qb * 512:(qb + 1) * 512], ps[:],
                                         AFT.Exp, scale=1.0 / 32.0)
            # Small constant loads, hard-anchored behind a phase-2 eviction:
            # the Tile scheduler hoists dep-free DMAs to the front of their
            # queue, where they steal DMA-wire slots from the critical x/W
            # streams (each stolen slot delays a wk chunk whose +900ns
            # sem-prop gates the projection j-loop).  An explicit dep holds
            # them until the wire's idle mid-kernel window.
            from concourse.tile import add_dep_helper

            anchor = ev_state["last"]
            for h in (nc.gpsimd.dma_start(ones_sb[:], ones1[:]),
                      nc.gpsimd.dma_start(id_sb[:], ident[:]),
                      nc.gpsimd.dma_start(wo_a[:], woTa[:])):
                add_dep_helper(h.ins, anchor.ins, True,
                               "hold const load until DMA-quiet window")

            v2t_tiles = {}

            def v_dst(hp):
                tau = hp >> 1
                if tau not in v2t_tiles:
                    v2t_tiles[tau] = v2t_pool.tile([128, 2048], BF16,
                                                   name=f"v2t_{tau}", tag="v2t")
                return v2t_tiles[tau]

            # seam bridge: V-proj (j=0, bb=0) from psV0 -- its psum never
            # touches psA/psB, so the PE crosses the pool swap without a stall
            ps0 = psV0.tile([128, 512], F32, name="ps0", tag="aux", bufs=1)
            for t in range(8):
                nc.tensor.matmul(ps0[:], w_v[:, 0, t], vx_tiles[0][:, t, :],
                                 start=(t == 0), stop=(t == 7))
            scatter_evict(v_dst, ps0[:], 0, 0, 512, 512, ("v", "s"))

            stkPS.close()
            stkKQ.close()

            # phases 4-6 share one PSUM pool (tags: vps 3 banks, pst 3,
            # pa 2) so there is no drain between V, AV and the output matmuls.
            stkV = ExitStack()
            pV = stkV.enter_context(tc.tile_pool(name="pV", bufs=1))
            V2 = pV.tile([128, 16, 1024], BF16, name="V2")
            with ExitStack() as ctx4:
                psB = ctx4.enter_context(tc.tile_pool(name="psB", bufs=4, space="PSUM"))
                pR = ctx4.enter_context(tc.tile_pool(name="pR", bufs=1))
                repT2 = pR.tile([128, 16, 4, 128], BF16, name="repT2")
                scratch = ctx4.enter_context(tc.tile_pool(name="scratch", bufs=4))
                ostp = ctx4.enter_context(tc.tile_pool(name="ostp", bufs=3))
                rzp = ctx4.enter_context(tc.tile_pool(name="rzp", bufs=4))

                def proj_v(jg):
                    for j in (jg, jg + 2, jg + 4, jg + 6):
                        for bb in range(4):
                            if j == 0 and bb == 0:
                                continue  # bridged from psV0 at the seam
                            ps = psB.tile([128, 512], F32, name="ps", tag="vps", bufs=5)
                            for t in range(8):
                                nc.tensor.matmul(ps[:], w_v[:, j, t], vx_tiles[bb][:, t, :],
                                                 start=(t == 0), stop=(t == 7))
                            scatter_evict(v_dst, ps[:], j, bb * 512, 512, 512,
                                          ("v", "s"))

                def transp_v(jg):
                    # XBAR DMA transpose: writes V2 directly, no PE/ACT/DVE work
                    for tau in range(4 * jg, 4 * jg + 4):
                        vt = v2t_tiles.pop(tau)
                        nc.sync.dma_start(V2[:, :, tau * 128:(tau + 1) * 128], vt[:],
                                          transpose=True)

                # phase 5/6 state + emitters
                pairs, rzs = {}, {}
                ost_state = {}

                def p5_z(qs):
                    zp = psV0.tile([128, 4], F32, name="zp", tag="aux", bufs=1)
                    for kt in range(16):
                        nc.tensor.matmul(zp[:], expS[:, kt, qs * 128:(qs + 1) * 128],
                                         ones_sb[:], start=(kt == 0), stop=(kt == 15))
                    rz = rzp.tile([128, 1], F32, name="rz", tag="rz")
                    nc.vector.reciprocal(rz[:], zp[:, 0:1])
                    rzs[qs] = rz

                def p5_av(qs, df, side=None, pops=None):
                    par, r0q = qs & 1, qs >> 1
                    if r0q not in pairs:
                        pairs[r0q] = scratch.tile([128, 2048], BF16, name="rep_pair",
                                                  tag="scr")
                    rep_pair = pairs[r0q]
                    pa = psB.tile([128, 512], F32, name="pa", tag="pa", bufs=2)
                    for kt in range(16):
                        nc.tensor.matmul(pa[:], expS[:, kt, qs * 128:(qs + 1) * 128],
                                         V2[:, kt, df * 512:(df + 1) * 512],
                                         start=(kt == 0), stop=(kt == 15))
                        # delay pops so the first po never waits on the XBAR
                        # repT2 transpose latency (in-order PE queue)
                        npop = (0 if df == 0 else 2) if pops is None else pops[kt]
                        for _ in range(npop):
                            if side:
                                side.popleft()()
                    # interleaved dest: col = d*2 + parity
                    nc.scalar.activation(
                        rep_pair[:, df * 1024 + par:df * 1024 + par + 1023:2], pa[:],
                        AFT.Copy, scale=rzs[qs][:])

                def emit_pair_transposes(r0q):
                    rp = pairs.pop(r0q)
                    # split halves on two queues: first half (consumed
                    # first by phase 6) lands ~1us earlier
                    nc.sync.dma_start(repT2[:, 0:8, r0q, :], rp[:, 0:1024],
                                      transpose=True)
                    nc.scalar.dma_start(repT2[:, 8:16, r0q, :], rp[:, 1024:2048],
                                        transpose=True)

                def transp_batch(rp, h4):
                    # final pair: PE transposes have ~2.5us less latency into
                    # the tail than the XBAR path, and keep the PE warm
                    prt_t = psV0.tile([128, 4, 128], BF16, name="prt_t",
                                     tag="aux", bufs=1)
                    for i in range(4):
                        hp16 = 4 * h4 + i
                        nc.tensor.transpose(prt_t[:, i],
                                            rp[:, hp16 * 128:(hp16 + 1) * 128],
                                            id_sb[:])
                    evict(repT2[:, 4 * h4:4 * h4 + 4, 3, :], prt_t[:], ("v", "s"))

                def p6_store(ost, r0, hp_first, nb, eng):
                    # one DMA for nb row tiles (rows 512 apart, same r0)
                    base = hp_first * 512
                    dst = o_part[base:base + nb * 512, :].rearrange(
                        "(f r) c -> r f c", r=512)[r0 * 128:r0 * 128 + 128]
                    eng.dma_start(dst, ost[:, 0:nb])

                def p6_half(r0, hp16, half, tail):
                    nb = 2 if tail else 4
                    bi = hp16 % nb
                    if bi == 0 and half == 0:
                        ost_state["t"] = (
                            ostp.tile([128, 2, 1024], BF16, name="ost2",
                                      tag="ost2", bufs=4) if tail else
                            ostp.tile([128, 4, 1024], BF16, name="ost", tag="ost"))
                    ost = ost_state["t"]
                    po = psB.tile([128, 512], F32, name="po", tag="vps", bufs=5)
                    nc.tensor.matmul(po[:], repT2[:, hp16, r0, :],
                                     wo_a[:, half * 512:(half + 1) * 512],
                                     start=True, stop=True)
                    dst = ost[:, bi, half * 512:(half + 1) * 512]
                    if tail:
                        evict(dst, po[:], ("v", "s"))
                    elif half == 0:
                        nc.scalar.copy(dst, po[:])
                    else:
                        nc.vector.tensor_copy(dst, po[:])
                    if half == 1:
                        if tail:
                            # single-tile stores, alternating queues: each
                            # tile hits the (saturated) wire as soon as its
                            # own evicts land, and the final store is a short
                            # 728ns transfer instead of a 2-tile 1456ns one
                            # final four tiles all on sync: the SWDGE gen
                            # (1038+650ns) is the late bound at the very end,
                            # HWDGE (625+650ns) keeps the wire fed
                            p6_store(ost[:, bi:bi + 1], r0, hp16, 1,
                                     nc.sync)
                        elif bi % 2 == 1:
                            # store each half-batch as soon as its 2 tiles are
                            # evicted (the wire slot starts ~1.3us earlier than
                            # a 4-tile store); late halves ride the SWDGE
                            # (gpsimd) queue: a store whose evicts lag parks
                            # its queue's gen stage, and on the sync queue that
                            # head-of-line blocks the next pair's transposes
                            eng = nc.sync if hp16 < 8 else nc.gpsimd
                            p6_store(ost[:, bi - 1:bi + 1], r0, hp16 - 1, 2,
                                     eng)

                def phase6_closures(r0):
                    from collections import deque

                    out = deque()
                    for hp16 in range(16):
                        for half in range(2):
                            out.append(
                                lambda r0=r0, hp16=hp16, half=half:
                                p6_half(r0, hp16, half, False))
                    return out

                def emit_phase6_r0(r0, h_lo=0, h_hi=16, tail=False):
                    for hp16 in range(h_lo, h_hi):
                        for half in range(2):
                            p6_half(r0, hp16, half, tail)

                # ---- emission: phase 4 with qs 0/1 df0 AV interleaved ----
                proj_v(0)
                transp_v(0)
                proj_v(1)
                p5_z(0)
                p5_av(0, 0)
                transp_v(1)
                stkVT.close()
                stkW.close()
                p5_z(1)
                p5_av(1, 0)
                p5_av(0, 1)
                p5_av(1, 1)
                emit_pair_transposes(0)
                pending = 0

                from collections import deque

                side = deque()
                for qs in range(2, 6):
                    p5_z(qs)
                    par, r0q = qs & 1, qs >> 1
                    if par == 0:
                        side.extend(phase6_closures(pending))
                        pending = None
                    p5_av(qs, 0, side)
                    p5_av(qs, 1, side)
                    while side:
                        side.popleft()()
                    if par == 1:
                        emit_pair_transposes(r0q)
                        pending = r0q

                # ---- final pair (qs 6,7): the whole tail is store-wire and
                # evict bound (r0=3's 4MB of o_part rides one DMA wire, each
                # po psum needs an ACT/DVE evict), so the goal is to start
                # r0=3's evict+store stream as early as the data deps allow:
                #  * hp0-7 (df0 cols) transposes + out-proj pop inside av(6,1)
                #  * av(7,1) runs as four 128-col chains; each chunk's hp
                #    pair transposes + out-proj pipeline into the next chunk
                p5_z(6)
                side.extend(phase6_closures(pending))
                pending = None
                p5_av(6, 0, side)
                p5_z(7)
                p5_av(7, 0, side,
                      pops=[2, 2, 2, 2, 2, 2, 2, 2, 2, 2, 2, 2, 2, 2, 2, 0])
                while side:                # 4 left; covers scale(7,0) latency
                    side.popleft()()
                rp3 = pairs[3]
                # df1 stretch: BOTH qs chunked 4x128 and interleaved, so
                # each hp pair's transpose unlocks after just two 850ns chunk
                # chains instead of after a full 3.4us av(6,1); the hp8-15
                # evict+store supply starts ~2.5us earlier and the late wire
                # idles collapse.  hp0-7 units pop between chunks (their
                # transposes go first, gated only on scale(7,0)).
                transp_batch(rp3, 0)
                transp_batch(rp3, 1)
                uq = deque()
                for hp16 in range(8):
                    for half in range(2):
                        uq.append(
                            lambda hp16=hp16, half=half:
                            p6_half(3, hp16, half, True))
                sc_state = {"i": 0}

                def av1_chunk(qs, c):
                    par = qs & 1
                    pa = psB.tile([128, 512], F32, name="pa", tag="pa", bufs=2)
                    for kt in range(16):
                        nc.tensor.matmul(pa[:, 0:128],
                                         expS[:, kt, qs * 128:(qs + 1) * 128],
                                         V2[:, kt, 512 + c * 128:512 + (c + 1) * 128],
                                         start=(kt == 0), stop=(kt == 15))
                    base = 1024 + c * 256
                    # strict ACT/DVE alternation across the 8 chunk scales
                    sc_state["i"] += 1
                    if sc_state["i"] % 2 == 1:
                        nc.scalar.activation(rp3[:, base + par:base + par + 255:2],
                                             pa[:, 0:128], AFT.Copy,
                                             scale=rzs[qs][:])
                    else:
                        nc.vector.tensor_scalar_mul(
                            rp3[:, base + par:base + par + 255:2], pa[:, 0:128],
                            rzs[qs][:])

                def transp_pair(c):
                    # hp blocks 8+2c, 9+2c of the final pair
                    prt_t = psV0.tile([128, 4, 128], BF16, name="prt_t",
                                     tag="aux", bufs=1)
                    for i in range(2):
                        hp16 = 8 + 2 * c + i
                        nc.tensor.transpose(prt_t[:, i],
                                            rp3[:, hp16 * 128:(hp16 + 1) * 128],
                                            id_sb[:])
                    evict(repT2[:, 8 + 2 * c:10 + 2 * c, 3, :], prt_t[:, 0:2],
                          ("v", "s"))

                for c in range(4):
                    av1_chunk(6, c)
                    for _ in range(2):
                        if uq:
                            uq.popleft()()
                    av1_chunk(7, c)
                    for _ in range(2):
                        if uq:
                            uq.popleft()()
                    transp_pair(c)
                    if c >= 1:
                        for hp16 in (6 + 2 * c, 7 + 2 * c):
                            for half in range(2):
                                p6_half(3, hp16, half, True)
                pairs.pop(3)
                for hp16 in (14, 15):
                    for half in range(2):
                        p6_half(3, hp16, half, True)
            stkV.close()
            stkV0.close()
            stkI.close()
            stkE.close()

    nc.compile()
    return nc


def _bf16(x):
    import ml_dtypes

    return x.astype(ml_dtypes.bfloat16)


def _host_inputs(k, q, v, W_k, W_q, W_v, W_o):
    """Per-core input maps. Core c: bp = c//2 (shuffled batch), qh = c%2."""
    f32 = np.float32

    def xlay(xw):
        # xw [rows, 1024 feats] -> xT [1024, rows] -> [p][t][c]
        xT = np.ascontiguousarray(xw.T, dtype=f32)
        return _bf16(np.ascontiguousarray(
            xT.reshape(8, 128, xT.shape[1]).transpose(1, 0, 2)))

    def wlay(W):
        # W^T [1024 in, 1024 out] -> [p][j][t][c]: W^T[t*128+p, j*128+c]
        WT = np.ascontiguousarray(W.T, dtype=f32)
        arr = WT.reshape(8, 128, 8, 128).transpose(1, 2, 0, 3)
        return _bf16(np.ascontiguousarray(arr))

    W_oT = np.ascontiguousarray(W_o.T, dtype=f32)
    wks, wqs, wvs = wlay(W_k), wlay(W_q), wlay(W_v)
    ones = _bf16(np.ones((128, 4), dtype=f32))
    identb = _bf16(np.eye(128, dtype=f32))
    in_maps = []
    for c in range(8):
        bp, qh = c // 2, c % 2
        kw = k[:, 512 * bp:512 * (bp + 1), :].reshape(2048, 1024)
        vw = v[:, 512 * bp:512 * (bp + 1), :].reshape(2048, 1024)
        qw = q[:, 512 * bp + 256 * qh:512 * bp + 256 * (qh + 1), :].reshape(1024, 1024)
        h0 = 4 * bp + 2 * qh
        wo_nat = W_oT[h0 * 64:h0 * 64 + 128, :]
        wo_nat = np.ascontiguousarray(
            wo_nat.reshape(2, 64, 1024).transpose(1, 0, 2).reshape(128, 1024))
        in_maps.append({
            "kx": xlay(kw), "vx": xlay(vw), "qx": xlay(qw),
            "wk": wks, "wq": wqs, "wv": wvs,
            "woTa": _bf16(wo_nat),
            "ones1": ones, "ident": identb,
        })
    return in_maps


def kernel(k, q, v, W_k, W_q, W_v, W_o, _want_trace=False):
    from concourse.bass_utils import run_bass_kernel_spmd

    if "nc" not in _CACHE:
        _CACHE["nc"] = _build_program()
    nc = _CACHE["nc"]

    in_maps = _host_inputs(np.asarray(k), np.asarray(q), np.asarray(v),
                           np.asarray(W_k), np.asarray(W_q), np.asarray(W_v),
                           np.asarray(W_o))
    res = run_bass_kernel_spmd(nc, in_maps, core_ids=list(range(8)),
                               trace=_want_trace)
    out = np.zeros((8192, 1024), dtype=np.float32)
    for r in res.results:
        out += r["o_part"].astype(np.float32)
    # rows are (h', r0, b, hi); real s = hi*64 + r0*16 + h'
    out = out.reshape(16, 4, 4, 32, D).transpose(2, 3, 1, 0, 4).reshape(B, S, D)
    if _want_trace:
        _CACHE["last_result"] = res
    return out

